# revision 25
# baseline (speedup 1.0000x reference)
"""MLA (DeepSeek-style multi-head latent attention) Bass kernel for 8 trn2 NeuronCores.

v5 design (evolved from v3/v4; see git-less history in transcript):
- Tensor-parallel over heads (2 heads/core); stage 0 (low-rank A
  projections) sequence-sharded (256 tokens/core) in [channel, token]
  layout; raw q-latents + per-token q-norm scale row AllGathered (norm
  scale commutes through wq_b, applied on the consumer side); kv latents
  normalized + rope'd at the source, gathered second.
- Q gather is triggered first and the kv collective carries an explicit
  dependency on it so the CC engine processes q before kv (Tile's
  scheduler otherwise reorders them); the q consumers (rcqb broadcast,
  q projection) are the long pole and start as soon as the q gather
  lands, while kn/v projections + q-rope fill the AG_kv window.
- No warmup collective: the cross-core launch-skew barrier (~45us)
  binds to the first CC op and ends at a fixed absolute time; a warmup
  op only added CC-queue latency in front of the data gathers (v4).
- Stage-0 operand DMAs chunked (hid in 4 slices, A-weights per c-tile)
  so the first matmul issues ~8us in.
- Attention: flat software pipeline across all 8 (sb,h) blocks — the
  next block's first score tile issues before this block's psx tail, so
  the in-order PE queue never stalls on the exp round-trip and the HAM
  clock stays at 8/8 (measured: 63ns inter-burst gaps in v4).
- Softmax denominator off the PE: ex tiles are summed on the DVE into
  exsum (f32) and a single ones-matmul per block reduces the partition
  axis (v3/v4 spent 128 matmuls = ~32us of PE on this).
- reciprocal_approx_fast for 1/den (full DVE reciprocal was ~3us per
  block); wo partials bf16 (halves out DMA); wo psum->sbuf copies
  alternate vector/scalar so neither engine serializes the wo tail.
- Host-side (free) prep: weight transposes into partition-major layouts;
  q_norm/kv_norm and SOFTMAX_SCALE folded into wq_b/wkv_b; rope sign
  folding.
"""

import math
import sys

import numpy as np

for _p in ("/opt/trn_rl_repo", "/root/.axon_site/_ro/trn_rl_repo"):
    if _p not in sys.path:
        sys.path.append(_p)

B, S, H = 1, 2048, 2048
NH = 16
Q_LORA, KV_LORA = 1536, 512
D_NOPE, D_ROPE, D_V = 128, 64, 128
D_QK = D_NOPE + D_ROPE
ROPE_FACTOR, MSCALE = 4.0, 1.0
SOFTMAX_SCALE = D_QK ** -0.5 * (0.1 * MSCALE * math.log(ROPE_FACTOR) + 1.0) ** 2
EPS = 1e-6

NCORES = 8
SSH = S // NCORES          # 256 tokens per core in stage 0
NQT = 12                   # q-latent channel tiles (1536/128)
NKT = 5                    # kv-latent tiles: 4x128 kv_c + 1 (64 kpe + 64 zero)

_CACHE = {}


def _build(has_mask: bool):
    import concourse.bacc as bacc
    import concourse.mybir as mybir
    import concourse.tile as tile
    from concourse.bass import _add_dep_helper

    f32 = mybir.dt.float32
    bf16 = mybir.dt.bfloat16
    AF = mybir.ActivationFunctionType
    OP = mybir.AluOpType

    nc = bacc.Bacc("TRN2", target_bir_lowering=False, debug=False,
                   num_devices=NCORES)

    # ---- external inputs (all partition-major, contiguous) ----
    hidp = nc.dram_tensor("hidp", [128, 16, SSH], bf16, kind="ExternalInput")
    aq_p = nc.dram_tensor("aq_p", [128, NQT, 16, 128], bf16,
                          kind="ExternalInput")
    akv_p = nc.dram_tensor("akv_p", [128, NKT, 16, 128], bf16,
                           kind="ExternalInput")
    cos_sh = nc.dram_tensor("cos_sh", [64, SSH], bf16, kind="ExternalInput")
    sins_sh = nc.dram_tensor("sins_sh", [64, SSH], bf16, kind="ExternalInput")
    cos2 = nc.dram_tensor("cos2", [128, S], bf16, kind="ExternalInput")
    sin2s = nc.dram_tensor("sin2s", [128, S], bf16, kind="ExternalInput")
    wqbp = nc.dram_tensor("wqbp", [128, NQT, 384], bf16, kind="ExternalInput")
    wkvbp = nc.dram_tensor("wkvbp", [128, 4, 512], bf16, kind="ExternalInput")
    wop = nc.dram_tensor("wop", [128, 2, S], bf16, kind="ExternalInput")
    ones_c = nc.dram_tensor("ones_c", [128, 1], bf16, kind="ExternalInput")
    ones_r = nc.dram_tensor("ones_r", [1, 128], bf16, kind="ExternalInput")
    if has_mask:
        maskT = nc.dram_tensor("maskT", [S, S], f32, kind="ExternalInput")
    out = nc.dram_tensor("out", [S, S], bf16, kind="ExternalOutput")

    NBT = 18
    bounce_q = nc.dram_tensor("bounce_q", [128, 13, SSH], bf16)
    gath_q = nc.dram_tensor("gath_q", [NCORES, 128, 13, SSH], bf16,
                            addr_space="Shared")
    bounce_kv = nc.dram_tensor("bounce_kv", [128, NKT, SSH], bf16)
    gath_kv = nc.dram_tensor("gath_kv", [NCORES, 128, NKT, SSH], bf16,
                             addr_space="Shared")

    RG = [list(range(NCORES))]

    def mm(ps, lhsT, rhs, start, stop):
        nc.tensor.matmul(ps, lhsT, rhs, start=start, stop=stop)

    from contextlib import ExitStack
    with tile.TileContext(nc) as tc, ExitStack() as _st:
        constp = _st.enter_context(tc.tile_pool(name="const", bufs=1))
        ones_col = constp.tile([128, 1], bf16)
        nc.sync.dma_start(ones_col[:], ones_c.ap())
        ones_row = constp.tile([1, 128], bf16)
        nc.sync.dma_start(ones_row[:], ones_r.ap())
        eps_sb = constp.tile([1, 1], f32)
        nc.any.memset(eps_sb[:], EPS)
        ones_full = constp.tile([128, 128], bf16)
        nc.any.memset(ones_full[:], 1.0)
        # stage-1 weights (tiles declared here; DMAs issued after the
        # stage-0 operand loads so stage 0 starts ASAP)
        wqb_sb = constp.tile([128, NQT, 384], bf16)
        wkvb_sb = constp.tile([128, 4, 512], bf16)
        wo_sb = constp.tile([128, 2, S], bf16)
        cos2_sb = constp.tile([128, S], bf16)
        sin2s_sb = constp.tile([128, S], bf16)

        # ---------------- stage 0: latents for own 256 tokens, [c, s] layout
        with tc.tile_pool(name="s0", bufs=1) as s0p, \
             tc.tile_pool(name="s0ps", bufs=3, space="PSUM") as s0ps, \
             tc.tile_pool(name="s0ss", bufs=1, space="PSUM") as s0ssp, \
             tc.tile_pool(name="s0pb", bufs=2, space="PSUM") as s0pb, \
             tc.tile_pool(name="s0sq", bufs=3) as s0sqp:
            # operand DMA staging: hid + first A-tiles get dedicated queues
            # (everything at once jams the queues and delays the first mm)
            hid_sb = s0p.tile([128, 16, SSH], bf16)
            for q8 in range(8):
                nc.sync.dma_start(hid_sb[:, 2 * q8:2 * q8 + 2, :],
                                  hidp.ap()[:, 2 * q8:2 * q8 + 2, :])
            aq_sb = s0p.tile([128, NQT, 16, 128], bf16)
            for qhb in range(4):
                nc.sync.dma_start(
                    aq_sb[:, 0, 4 * qhb:4 * qhb + 4, :],
                    aq_p.ap()[:, 0, 4 * qhb:4 * qhb + 4, :])
            for ct in range(1, NQT):
                for hhb in range(2):
                    nc.sync.dma_start(
                        aq_sb[:, ct, 8 * hhb:8 * hhb + 8, :],
                        aq_p.ap()[:, ct, 8 * hhb:8 * hhb + 8, :])
            akv_sb = s0p.tile([128, NKT, 16, 128], bf16)
            for ct in range(NKT):
                nc.sync.dma_start(akv_sb[:, ct], akv_p.ap()[:, ct])
            nc.sync.dma_start(wqb_sb[:], wqbp.ap())
            nc.sync.dma_start(wkvb_sb[:], wkvbp.ap())
            nc.sync.dma_start(wo_sb[:], wop.ap())
            nc.sync.dma_start(cos2_sb[:], cos2.ap())
            nc.sync.dma_start(sin2s_sb[:], sin2s.ap())

            ss_hq = s0ssp.tile([128, SSH], f32)
            ss_kv = s0ssp.tile([128, SSH], f32)

            # HAM pre-warm: ~7us of throwaway matmuls while the operand DMAs
            # land, so the PE clock is at full rate when stage 0 starts
            with tc.tile_pool(name="warm", bufs=1, space="PSUM") as warmp:
                wps = warmp.tile([128, 128], f32)
                for _ in range(95):
                    mm(wps, ones_full, ones_full, True, True)

            # --- raw q-latent tiles (q gather is the long pole: first) ---
            lat = s0p.tile([128, 13, SSH], bf16)
            for ct in range(NQT):
                ps = s0ps.tile([128, SSH], f32, tag="s0ps")
                for hb in range(16):
                    mm(ps, aq_sb[:, ct, hb, :], hid_sb[:, hb, :],
                       hb == 0, hb == 15)
                with nc.allow_low_precision(reason="bf16 latents"):
                    nc.vector.tensor_copy(lat[:, ct, :], ps[:])
                sq = s0sqp.tile([128, SSH], bf16, tag="s0sq")
                nc.scalar.activation(sq[:], ps[:], AF.Square)
                mm(ss_hq, ones_full, sq, ct == 0, ct == NQT - 1)
                # stream the bounce write per 3-tile group so only the last
                # group + rc row sit on the pre-trigger critical path
                if ct % 3 == 2:
                    nc.sync.dma_start(bounce_q.ap()[:, ct - 2:ct + 1, :],
                                      lat[:, ct - 2:ct + 1, :])

            # q rms scale row -> rides in tile 12
            sq_hq = s0p.tile([1, SSH], f32)
            nc.scalar.activation(sq_hq[:], ss_hq[0:1, :], AF.Sqrt,
                                 bias=eps_sb[:], scale=1.0 / Q_LORA)
            nc.any.memset(lat[:, 12, :], 0.0)
            with nc.allow_low_precision(reason="bf16 rms scale"):
                nc.vector.reciprocal(lat[0:1, 12, :], sq_hq[:])
            nc.sync.dma_start(bounce_q.ap()[:, 12:13, :], lat[:, 12:13, :])
            cc_q = nc.gpsimd.collective_compute(
                "AllGather", OP.bypass, replica_groups=RG,
                ins=[bounce_q.ap().opt()], outs=[gath_q.ap().opt()])

            # --- kv-latent tiles: normalized at source (+rope'd kpe) ---
            raw_kv = s0p.tile([128, NKT, SSH], bf16)
            for ct in range(NKT):
                ps = s0ps.tile([128, SSH], f32, tag="s0ps")
                for hb in range(16):
                    mm(ps, akv_sb[:, ct, hb, :], hid_sb[:, hb, :],
                       hb == 0, hb == 15)
                with nc.allow_low_precision(reason="bf16 latents"):
                    nc.vector.tensor_copy(raw_kv[:, ct, :], ps[:])
                if ct < 4:
                    sq = s0sqp.tile([128, SSH], bf16, tag="s0sq")
                    nc.scalar.activation(sq[:], ps[:], AF.Square)
                    mm(ss_kv, ones_full, sq, ct == 0, ct == 3)

            sq_kv = s0p.tile([1, SSH], f32)
            nc.scalar.activation(sq_kv[:], ss_kv[0:1, :], AF.Sqrt,
                                 bias=eps_sb[:], scale=1.0 / KV_LORA)
            rc_kv = s0p.tile([1, SSH], bf16)
            with nc.allow_low_precision(reason="bf16 rms scale"):
                nc.vector.reciprocal(rc_kv[:], sq_kv[:])
            psb_kv = s0pb.tile([128, SSH], f32, tag="s0pb")
            mm(psb_kv, ones_row, rc_kv, True, True)
            bc_kv = s0p.tile([128, SSH], f32)
            nc.scalar.copy(bc_kv[:], psb_kv[:])

            lat_kv = s0p.tile([128, NKT, SSH], bf16)
            for ct in range(4):
                with nc.allow_low_precision(reason="bf16 latents"):
                    nc.vector.tensor_tensor(lat_kv[:, ct, :], raw_kv[:, ct, :],
                                            bc_kv[:], OP.mult)
            # k_pe rope (not normalized); rows [0:64) of tile 4; rows 64:128
            # are zero (zero rows of A) and just copied through.
            cs_sb = s0p.tile([64, SSH], bf16)
            nc.sync.dma_start(cs_sb[:], cos_sh.ap())
            sn_sb = s0p.tile([64, SSH], bf16)
            nc.sync.dma_start(sn_sb[:], sins_sh.ap())
            t1 = s0p.tile([64, SSH], bf16)
            nc.vector.tensor_tensor(t1[:], raw_kv[0:64, 4, :], cs_sb[:],
                                    OP.mult)
            rsw = s0p.tile([64, SSH], bf16)
            nc.sync.dma_start(rsw[0:32], raw_kv[32:64, 4, :])
            nc.sync.dma_start(rsw[32:64], raw_kv[0:32, 4, :])
            t2 = s0p.tile([64, SSH], bf16)
            nc.vector.tensor_tensor(t2[:], rsw[:], sn_sb[:], OP.mult)
            nc.vector.tensor_tensor(lat_kv[0:64, 4, :], t1[:], t2[:], OP.add)
            nc.vector.tensor_copy(lat_kv[64:128, 4, :], raw_kv[64:128, 4, :])
            nc.sync.dma_start(bounce_kv.ap(), lat_kv[:])
            cc_kv = nc.gpsimd.collective_compute(
                "AllGather", OP.bypass, replica_groups=RG,
                ins=[bounce_kv.ap().opt()], outs=[gath_kv.ap().opt()])
            # force CC processing order: q gather strictly before kv gather
            _add_dep_helper(cc_kv.ins, cc_q.ins, True,
                            "kv gather after q gather on the CC stream")

        # ---------------- stage 1: per-head projections + attention + wo
        with tc.tile_pool(name="s1", bufs=1) as s1p:
            # gathered latents -> SBUF, [c, r, ct, s]; q chunked so the
            # rcqb broadcast + q-proj start on the first chunks.
            # per-rank loads: src gath_q[r] and dst g_sb[:, r, 0:13, :] are
            # both contiguous (6.6KB/partition runs), and q-proj consumes
            # rank-pairs in sc order, so sc=0 starts after 2 of 8 DMAs.
            g_sb = s1p.tile([128, NCORES, NBT, SSH], bf16)
            for r in range(NCORES):
                nc.sync.dma_start(g_sb[:, r, 0:13, :], gath_q.ap()[r])
            for r in range(NCORES):
                nc.sync.dma_start(g_sb[:, r, 13:18, :], gath_kv.ap()[r])

            # ~50us of throwaway matmuls spanning the q-gather wait: the PE
            # has provably nothing to do (every stage-1 consumer needs the
            # gather), and >3us of idle drops the HAM clock to 4/8 with a
            # slow 13/16 recovery ramp that taxes q-proj + early attention.
            with tc.tile_pool(name="fill", bufs=1, space="PSUM") as fillp:
                fps = fillp.tile([128, 512], f32)
                for _ in range(240):
                    mm(fps, ones_full, cos2_sb[:, 0:512], True, True)

            p1ctx = tc.tile_pool(name="p1ps", bufs=3, space="PSUM")
            p1ps = p1ctx.__enter__()
            p1bc = tc.tile_pool(name="p1bc", bufs=1, space="PSUM")
            p1bcp = p1bc.__enter__()

            # q-norm scale broadcast [128, S] from the gathered rc row
            rcqb = s1p.tile([128, S], f32)
            for sc in range(4):
                psb = p1bcp.tile([128, 512], f32, tag="p1bc")
                mm(psb, ones_row, g_sb[0:1, 2 * sc:2 * sc + 2, 12, :],
                   True, True)
                nc.scalar.copy(rcqb[:, sc * 512:(sc + 1) * 512], psb[:])

            # q projection: m=0 qn0(h0 nope), m=1 qt1(h0+h1 rope), m=2 qn1;
            # consumer-side per-token q-norm scale applied on psum read-out
            qn0 = s1p.tile([128, S], bf16)
            qt1 = s1p.tile([128, S], bf16)
            qn1 = s1p.tile([128, S], bf16)
            qdst = (qn0, qt1, qn1)
            qt1r = s1p.tile([128, S], bf16)
            qr1 = s1p.tile([128, S], bf16)
            # sc-major so consumption follows the per-rank load arrival
            # order (sc uses ranks 2sc, 2sc+1)
            for sc in range(4):
                for m in range(3):
                    ps = p1ps.tile([128, 512], f32, tag="p1ps")
                    for cc in range(NQT):
                        mm(ps, wqb_sb[:, cc, m * 128:(m + 1) * 128],
                           g_sb[:, 2 * sc:2 * sc + 2, cc, :],
                           cc == 0, cc == NQT - 1)
                    with nc.allow_low_precision(reason="bf16 q"):
                        nc.vector.tensor_tensor(
                            qdst[m][:, sc * 512:(sc + 1) * 512], ps[:],
                            rcqb[:, sc * 512:(sc + 1) * 512], OP.mult)

            # rope on q (qt1 rows 0:64 = h0 rope, 64:128 = h1 rope) on DVE
            # while the PE moves on to kn/v below.
            with tc.tile_pool(name="rope", bufs=1) as rp:
                tmp = rp.tile([128, S], bf16)
                for b in (0, 64):
                    nc.sync.dma_start(tmp[b:b + 32], qt1[b + 32:b + 64])
                    nc.sync.dma_start(tmp[b + 32:b + 64], qt1[b:b + 32])
                nc.vector.tensor_tensor(qt1r[:], qt1[:], cos2_sb[:], OP.mult)
                nc.vector.tensor_tensor(tmp[:], tmp[:], sin2s_sb[:], OP.mult)
                nc.vector.tensor_tensor(qt1r[:], qt1r[:], tmp[:], OP.add)
                nc.sync.dma_start(qr1[0:64, :], qt1r[64:128])
                nc.sync.dma_start(qr1[64:128, :], qt1r[0:64])

            # kn projection per head: kn[kh] = [d_nope=128, S]
            kn0 = s1p.tile([128, S], bf16)
            kn1 = s1p.tile([128, S], bf16)
            kn = (kn0, kn1)
            for kh in range(2):
                for sc in range(4):
                    ps = p1ps.tile([128, 512], f32, tag="p1ps")
                    for cc in range(4):
                        mm(ps, wkvb_sb[:, cc, kh * 128:(kh + 1) * 128],
                           g_sb[:, 2 * sc:2 * sc + 2, 13 + cc, :],
                           cc == 0, cc == 3)
                    with nc.allow_low_precision(reason="bf16 k"):
                        nc.vector.tensor_copy(
                            kn[kh][:, sc * 512:(sc + 1) * 512], ps[:])

            # v projection: vt[tb] = [t-chunk 128, 256 (v_h0|v_h1)]
            vt = s1p.tile([128, 16, 256], bf16)
            for tb in range(16):
                ps = p1ps.tile([128, 256], f32, tag="p1ps")
                for cc in range(4):
                    mm(ps, g_sb[:, tb // 2, 13 + cc,
                                (tb % 2) * 128:(tb % 2) * 128 + 128],
                       wkvb_sb[:, cc, 256:512], cc == 0, cc == 3)
                with nc.allow_low_precision(reason="bf16 v"):
                    nc.vector.tensor_copy(vt[:, tb, :], ps[:])

            p1bc.__exit__(None, None, None)
            p1ctx.__exit__(None, None, None)

            # attention + interleaved row-parallel wo, streaming over
            # s-blocks. Flat software pipeline across all 8 (sb,h) blocks.
            # Softmax denominator: DVE-accumulated exsum + one ones-matmul.
            with tc.tile_pool(name="xh", bufs=2) as xhp, \
                 tc.tile_pool(name="oot", bufs=6) as wootp:
              with tc.tile_pool(name="apss", bufs=2, space="PSUM") as apss, \
                 tc.tile_pool(name="apsx", bufs=1, space="PSUM") as apsx, \
                 tc.tile_pool(name="apsd", bufs=1, space="PSUM") as apsd, \
                 tc.tile_pool(name="wops", bufs=2, space="PSUM") as wops, \
                 tc.tile_pool(name="aex", bufs=3) as aexp, \
                 tc.tile_pool(name="asm", bufs=2) as asmp, \
                 tc.tile_pool(name="aes", bufs=2) as aesp, \
                 tc.tile_pool(name="amk", bufs=2) as amkp:
                xh0 = xhp.tile([128, 512], bf16)
                xh1 = xhp.tile([128, 512], bf16)
                xhb = (xh0, xh1)
                blocks = [(sb, h) for sb in range(4) for h in range(2)]

                def scores(bi, tp):
                    # two key-chunks (tb=2*tp, 2*tp+1) into one [128,1024]
                    # psum tile; a single exp for both
                    sb, h = blocks[bi]
                    sl = slice(sb * 512, (sb + 1) * 512)
                    qn_h = qn0 if h == 0 else qn1
                    qr_h = qt1r if h == 0 else qr1
                    pss = apss.tile([128, 1024], f32, tag="apss")
                    for half in range(2):
                        tb = 2 * tp + half
                        hsl = slice(half * 512, (half + 1) * 512)
                        mm(pss[:, hsl],
                           kn[h][:, tb * 128:(tb + 1) * 128],
                           qn_h[:, sl], True, False)
                        mm(pss[:, hsl],
                           g_sb[:, tb // 2, 17,
                                (tb % 2) * 128:(tb % 2) * 128 + 128],
                           qr_h[:, sl], False, True)
                        if has_mask:
                            mk = amkp.tile([128, 512], f32, tag="amk")
                            nc.sync.dma_start(
                                mk[:],
                                maskT.ap()[tb * 128:(tb + 1) * 128, sl])
                            nc.vector.tensor_tensor(
                                pss[:, hsl], pss[:, hsl], mk[:], OP.add)
                    ex = aexp.tile([128, 1024], bf16, tag="aex")
                    nc.scalar.activation(ex[:], pss[:], AF.Exp)
                    return ex

                ex_cur = scores(0, 0)
                for bi in range(8):
                    sb, h = blocks[bi]
                    sl = slice(sb * 512, (sb + 1) * 512)
                    psx = apsx.tile([128, 512], f32, tag="apsx")
                    psd = apsd.tile([128, 512], f32, tag="apsd")
                    exsum = aesp.tile([128, 512], f32, tag="exsum")
                    for tp in range(8):
                        if tp < 7:
                            ex_next = scores(bi, tp + 1)
                        elif bi < 7:
                            ex_next = scores(bi + 1, 0)
                        else:
                            ex_next = None
                        for half in range(2):
                            tb = 2 * tp + half
                            hsl = slice(half * 512, (half + 1) * 512)
                            mm(psx, vt[:, tb, h * 128:(h + 1) * 128],
                               ex_cur[:, hsl], tb == 0, tb == 15)
                        # denominator partials on the DVE (keeps the PE free)
                        if tp == 0:
                            nc.vector.tensor_tensor(
                                exsum[:], ex_cur[:, 0:512],
                                ex_cur[:, 512:1024], OP.add)
                        else:
                            nc.vector.tensor_tensor(
                                exsum[:], exsum[:], ex_cur[:, 0:512], OP.add)
                            nc.vector.tensor_tensor(
                                exsum[:], exsum[:], ex_cur[:, 512:1024],
                                OP.add)
                        ex_cur = ex_next
                    exsum_b = asmp.tile([128, 512], bf16, tag="exsb")
                    with nc.allow_low_precision(reason="bf16 denom"):
                        nc.vector.tensor_copy(exsum_b[:], exsum[:])
                    mm(psd, ones_full, exsum_b, True, True)
                    rdb = asmp.tile([128, 512], f32, tag="rdb")
                    nc.vector.reciprocal_approx_fast(rdb[:], psd[:])
                    with nc.allow_low_precision(reason="bf16 attn out"):
                        nc.vector.tensor_tensor(xhb[h][:], psx[:], rdb[:],
                                                OP.mult)
                    if h == 1 and sb < 3:
                        # wo for this s-block: partial out rows = all H,
                        # contraction over this core's 256 v-dims (2 heads)
                        for ht in range(16):
                            pso = wops.tile([128, 512], f32, tag="wops")
                            mm(pso, wo_sb[:, 0, ht * 128:(ht + 1) * 128],
                               xh0[:], True, False)
                            mm(pso, wo_sb[:, 1, ht * 128:(ht + 1) * 128],
                               xh1[:], False, True)
                            ot = wootp.tile([128, 512], bf16, tag="ot")
                            with nc.allow_low_precision(reason="bf16 out"):
                                if ht % 2 == 0:
                                    nc.vector.tensor_copy(ot[:], pso[:])
                                else:
                                    nc.scalar.copy(ot[:], pso[:])
                            nc.sync.dma_start(
                                out.ap()[ht * 128:(ht + 1) * 128, sl], ot[:])
                    elif h == 1:
                        # last s-block: remember its xh tiles; the wo runs
                        # below in a deep psum pool once the attention pools
                        # are released, so its 32 matmuls stream instead of
                        # trickling through 2 psum banks.
                        xh3 = (xh0, xh1)
              sl3 = slice(3 * 512, 4 * 512)
              with tc.tile_pool(name="wop2", bufs=6, space="PSUM") as wop2:
                for ht in range(16):
                    pso = wop2.tile([128, 512], f32, tag="w2")
                    mm(pso, wo_sb[:, 0, ht * 128:(ht + 1) * 128],
                       xh3[0][:], True, False)
                    mm(pso, wo_sb[:, 1, ht * 128:(ht + 1) * 128],
                       xh3[1][:], False, True)
                    ot = wootp.tile([128, 512], bf16, tag="ot")
                    with nc.allow_low_precision(reason="bf16 out"):
                        if ht % 2 == 0:
                            nc.vector.tensor_copy(ot[:], pso[:])
                        else:
                            nc.scalar.copy(ot[:], pso[:])
                    nc.sync.dma_start(
                        out.ap()[ht * 128:(ht + 1) * 128, sl3], ot[:])

    nc.compile()
    return nc


def _prep_inputs(hidden_states, cos, sin, attn_mask, wq_a, q_norm_w, wq_b,
                 wkv_a, kv_norm_w, wkv_b, wo, has_mask):
    import ml_dtypes
    bf16 = ml_dtypes.bfloat16
    c = np.ascontiguousarray

    hid = np.asarray(hidden_states, np.float32)[0]          # [S, H]
    hidT = hid.T                                            # [H, S]
    wqa = np.asarray(wq_a, np.float32)                      # [1536, H]
    wkva = np.asarray(wkv_a, np.float32)                    # [576, H]
    akv = np.vstack([wkva[:KV_LORA], wkva[KV_LORA:],
                     np.zeros((64, H), np.float32)])        # [640, H]
    # A-weights as per-c-tile lhsT tiles: [128, nct, 16, 128]
    A_q_T = wqa.T                                           # [H, 1536]
    aq_p = c(A_q_T.reshape(16, 128, NQT, 128)
             .transpose(1, 2, 0, 3).astype(bf16))
    A_kv_T = akv.T                                          # [H, 640]
    akv_p = c(A_kv_T.reshape(16, 128, NKT, 128)
              .transpose(1, 2, 0, 3).astype(bf16))

    cosT = np.asarray(cos, np.float32).T                    # [64, S]
    sinT = np.asarray(sin, np.float32).T
    sinTs = sinT.copy()
    sinTs[0:32] *= -1.0
    cos2 = c(np.concatenate([cosT, cosT], 0).astype(bf16))  # [128, S]
    sin2s = c(np.concatenate([sinTs, sinTs], 0).astype(bf16))

    wqb = np.asarray(wq_b, np.float32) * np.asarray(q_norm_w, np.float32)[None]
    wqb = wqb * SOFTMAX_SCALE                               # [3072, 1536]
    wkvb = (np.asarray(wkv_b, np.float32)
            * np.asarray(kv_norm_w, np.float32)[None])      # [4096, 512]
    wo_f = np.asarray(wo, np.float32)                       # [H, NH*D_V]

    qperm = np.r_[0:128, 128:192, 320:384, 192:320]
    kvperm = np.r_[0:128, 256:384, 128:256, 384:512]

    in_maps = []
    for r in range(NCORES):
        wqb_r = wqb[r * 384:(r + 1) * 384].T[:, qperm]      # [1536, 384]
        wkvb_r = wkvb[r * 512:(r + 1) * 512].T[:, kvperm]   # [512, 512]
        wo_r = wo_f[:, r * 256:(r + 1) * 256].T             # [256, H]
        m = {
            "hidp": c(hidT[:, r * SSH:(r + 1) * SSH]
                      .reshape(16, 128, SSH).transpose(1, 0, 2).astype(bf16)),
            "aq_p": aq_p,
            "akv_p": akv_p,
            "cos_sh": c(cosT[:, r * SSH:(r + 1) * SSH].astype(bf16)),
            "sins_sh": c(sinTs[:, r * SSH:(r + 1) * SSH].astype(bf16)),
            "cos2": cos2,
            "sin2s": sin2s,
            "wqbp": c(wqb_r.reshape(NQT, 128, 384)
                      .transpose(1, 0, 2).astype(bf16)),
            "wkvbp": c(wkvb_r.reshape(4, 128, 512)
                       .transpose(1, 0, 2).astype(bf16)),
            "wop": c(wo_r.reshape(2, 128, S).transpose(1, 0, 2).astype(bf16)),
            "ones_c": np.ones((128, 1), np.float32).astype(bf16),
            "ones_r": np.ones((1, 128), np.float32).astype(bf16),
        }
        if has_mask:
            m["maskT"] = c(np.asarray(attn_mask, np.float32).T)
        in_maps.append(m)
    return in_maps


def kernel(**inputs):
    from concourse.bass_utils import run_bass_kernel_spmd

    has_mask = bool(np.any(np.asarray(inputs["attn_mask"])))
    if has_mask not in _CACHE:
        _CACHE[has_mask] = _build(has_mask)
    nc = _CACHE[has_mask]

    in_maps = _prep_inputs(has_mask=has_mask, **inputs)
    res = run_bass_kernel_spmd(nc, in_maps, list(range(NCORES))).results
    return combine([res[r]["out"] for r in range(NCORES)])


def combine(parts):
    """Sum per-core [H, S] partials (bf16) and return [B, S, H] f32."""
    full = np.zeros((H, S), np.float32)
    for p in parts:
        full += np.asarray(p, np.float32)
    return np.ascontiguousarray(full.T).reshape(B, S, H)
